# revision 8
# baseline (speedup 1.0000x reference)
"""ANOVA kernel (order 3) on 8 TRN2 NeuronCores.

Math: out[b] = sum_e e3(x[b, :, e]) where e3 is the 3rd elementary
symmetric polynomial over the field axis. Via Newton's identities:
    e3 = (p1^3 - 3*p1*p2 + 2*p3) / 6,   p_k = sum_f x^k
so the kernel is: elementwise x^2 (ScalarE), x^3 (VectorE), three
field-axis reductions (TensorE matmuls with one-hot selector weights,
float32r), then a tiny fused finale.

Data parallel over batch: core c handles b in [1024*c, 1024*(c+1)).

Layout per core: tile tau covers 16 consecutive b. SBUF tile (128, 512):
partition p = b_hi*64 + f  (b_hi in {0,1}),  free n = j*64 + e  (j in [0,8)),
holding x[b0 + 2j + b_hi, f, e] with b0 = 16*tau. A matmul with one-hot
lhsT (col m = 2*tau + b_hi) accumulates each tile's field-sums into PSUM
rows 2*tau, 2*tau+1 — after 64 tiles the (128, 512) PSUM tensor holds the
per-(b, e) power sums for all 1024 b of the core.
"""

import sys

if "/opt/trn_rl_repo" not in sys.path:
    sys.path.insert(0, "/opt/trn_rl_repo")

import numpy as np

N_CORES = 8
B, F, E = 8192, 64, 64
B_PER_CORE = B // N_CORES  # 1024
J = 8                      # b-pairs per tile; matmul free dim = J*E = 512
FD = J * E                 # 512 (one PSUM bank)
TILES = B_PER_CORE // (2 * J)  # 64
SUPER = 4                  # tiles per superblock for big ACT/DVE instructions
N_SUPER = TILES // SUPER   # 16
SFD = FD * SUPER           # 2048

_cache = {}


def _make_g() -> np.ndarray:
    """One-hot selector weights G (128, 254), bf16.

    lhsT for tile tau is G[:, 126-2*tau : 254-2*tau]; then
    lhsT[k, m] = 1 iff m == 2*tau + (k // 64).
    """
    import ml_dtypes

    g = np.zeros((128, 254), dtype=ml_dtypes.bfloat16)
    for p in range(128):
        g[p, 126 + (p // 64)] = 1.0
    return g


def _build():
    import concourse.bass as bass
    import concourse.tile as tile
    from concourse import bacc, mybir

    nc = bacc.Bacc(
        "TRN2", target_bir_lowering=False, debug=False, num_devices=N_CORES
    )
    f32 = mybir.dt.float32
    bf16 = mybir.dt.bfloat16

    x_dram = nc.dram_tensor(
        "x", [B_PER_CORE, F, E], f32, kind="ExternalInput"
    ).ap()
    g_dram = nc.dram_tensor("g", [128, 254], bf16, kind="ExternalInput").ap()
    out_dram = nc.dram_tensor("out", [128, J], f32, kind="ExternalOutput").ap()

    # b = 16*t + 2*j + p  ->  partition (p f), free (k j e).
    # The composite partition index p*64+f maps to DRAM element offset
    # p*F*E + f*E = (p*64 + f)*64 — a uniform stride of 64, so one AP dim.
    def x_super_ap(s: int) -> bass.AP:
        return bass.AP(
            tensor=x_dram.tensor,
            offset=s * SUPER * 16 * F * E,
            ap=[
                [E, 128],           # partition: (b_hi, f), stride E=64
                [16 * F * E, SUPER],  # k: tile within superblock (16 b's)
                [2 * F * E, J],     # j: b-pair
                [1, E],             # e
            ],
        )

    with tile.TileContext(nc) as tc:
        with (
            tc.tile_pool(name="const", bufs=1) as const_pool,
            tc.tile_pool(name="xin", bufs=3) as x_pool,
            tc.tile_pool(name="xsq", bufs=2) as x2_pool,
            tc.tile_pool(name="xcu", bufs=2) as x3_pool,
            tc.tile_pool(name="acc", bufs=1, space="PSUM") as psum_pool,
            tc.tile_pool(name="tail", bufs=1) as tail_pool,
        ):
            g_sb = const_pool.tile([128, 254], bf16)
            nc.sync.dma_start(out=g_sb[:], in_=g_dram[:])

            p1 = psum_pool.tile([128, FD], f32)
            p2 = psum_pool.tile([128, FD], f32)
            p3 = psum_pool.tile([128, FD], f32)

            for s in range(N_SUPER):
                xb = x_pool.tile([128, SFD], bf16)
                # SWDGE cast-DMA: fp32 in DRAM -> bf16 in SBUF
                nc.gpsimd.dma_start(out=xb[:], in_=x_super_ap(s))
                x2b = x2_pool.tile([128, SFD], bf16)
                nc.scalar.square(x2b[:], xb[:])
                x3b = x3_pool.tile([128, SFD], bf16)
                nc.vector.tensor_mul(x3b[:], x2b[:], xb[:])
                for k in range(SUPER):
                    t = s * SUPER + k
                    lhsT = g_sb[:, 126 - 2 * t : 254 - 2 * t]
                    first = t == 0
                    last = t == TILES - 1
                    sl = slice(k * FD, (k + 1) * FD)
                    for psum, src in ((p1, xb), (p2, x2b), (p3, x3b)):
                        nc.tensor.matmul(
                            psum[:],
                            lhsT,
                            src[:, sl],
                            start=first,
                            stop=last,
                            skip_group_check=True,
                        )

            # finale: e3 = (p1^3 - 3 p1 p2 + 2 p3) / 6, summed over e
            t1 = tail_pool.tile([128, FD], f32)
            nc.scalar.square(t1[:], p1[:])  # p1^2
            u2 = tail_pool.tile([128, FD], f32)
            nc.vector.scalar_tensor_tensor(  # p1^2 - 3 p2
                u2[:], p2[:], -3.0, t1[:],
                op0=mybir.AluOpType.mult, op1=mybir.AluOpType.add,
            )
            u3 = tail_pool.tile([128, FD], f32)
            nc.vector.tensor_mul(u3[:], u2[:], p1[:])  # p1^3 - 3 p1 p2
            u5 = tail_pool.tile([128, FD], f32)
            nc.vector.scalar_tensor_tensor(  # + 2 p3
                u5[:], p3[:], 2.0, u3[:],
                op0=mybir.AluOpType.mult, op1=mybir.AluOpType.add,
            )
            red = tail_pool.tile([128, J], f32)
            nc.vector.reduce_sum(
                red[:],
                u5[:].rearrange("p (j e) -> p j e", j=J),
                axis=mybir.AxisListType.X,
            )
            outt = tail_pool.tile([128, J], f32)
            nc.vector.tensor_scalar_mul(outt[:], red[:], 1.0 / 6.0)
            nc.sync.dma_start(out=out_dram[:], in_=outt[:])

    nc.compile()
    return nc


def _get_nc():
    if "nc" not in _cache:
        _cache["nc"] = _build()
    return _cache["nc"]


def _unpermute(r: np.ndarray) -> np.ndarray:
    # r[(2*tau + b_hi), j] is the value for b_local = 16*tau + 2*j + b_hi
    return np.transpose(r.reshape(TILES, 2, J), (0, 2, 1)).reshape(-1)


def _run(x: np.ndarray, **kwargs):
    from concourse.bass_utils import run_bass_kernel_spmd

    nc = _get_nc()
    g = _make_g()
    shards = x.reshape(N_CORES, B_PER_CORE, F, E)
    in_maps = [
        {"x": np.ascontiguousarray(shards[c]), "g": g} for c in range(N_CORES)
    ]
    res = run_bass_kernel_spmd(nc, in_maps, core_ids=list(range(N_CORES)), **kwargs)
    out = np.concatenate(
        [_unpermute(np.asarray(res.results[c]["out"])) for c in range(N_CORES)]
    ).astype(np.float32)
    return out, res


def kernel(**inputs) -> np.ndarray:
    x = np.ascontiguousarray(np.asarray(inputs["x"], dtype=np.float32))
    assert x.shape == (B, F, E), x.shape
    out, _ = _run(x)
    return out


# revision 9
# speedup vs baseline: 1.5208x; 1.5208x over previous
"""ANOVA kernel (order 3) on 8 TRN2 NeuronCores.

Math: out[b] = sum_e e3(x[b, :, e]) where e3 is the 3rd elementary
symmetric polynomial over the field axis. Via Newton's identities:
    e3 = (p1^3 - 3*p1*p2 + 2*p3) / 6,   p_k = sum_f x^k
so the kernel is: elementwise x^2 (ScalarE), x^3 (VectorE), three
field-axis reductions (TensorE matmuls with one-hot selector weights,
float32r), then a tiny fused finale.

Data parallel over batch: core c handles b in [1024*c, 1024*(c+1)).

Layout per core: tile tau covers 16 consecutive b. SBUF tile (128, 512):
partition p = b_hi*64 + f  (b_hi in {0,1}),  free n = j*64 + e  (j in [0,8)),
holding x[b0 + 2j + b_hi, f, e] with b0 = 16*tau. A matmul with one-hot
lhsT (col m = 2*tau + b_hi) accumulates each tile's field-sums into PSUM
rows 2*tau, 2*tau+1 — after 64 tiles the (128, 512) PSUM tensor holds the
per-(b, e) power sums for all 1024 b of the core.
"""

import sys

if "/opt/trn_rl_repo" not in sys.path:
    sys.path.insert(0, "/opt/trn_rl_repo")

import numpy as np

N_CORES = 8
B, F, E = 8192, 64, 64
B_PER_CORE = B // N_CORES  # 1024
J = 8                      # b-pairs per tile; matmul free dim = J*E = 512
FD = J * E                 # 512 (one PSUM bank)
TILES = B_PER_CORE // (2 * J)  # 64
SUPER = 4                  # tiles per superblock for big ACT/DVE instructions
N_SUPER = TILES // SUPER   # 16
SFD = FD * SUPER           # 2048

_cache = {}


def _make_g() -> np.ndarray:
    """One-hot selector weights G (128, 254), bf16.

    lhsT for tile tau is G[:, 126-2*tau : 254-2*tau]; then
    lhsT[k, m] = 1 iff m == 2*tau + (k // 64).
    """
    import ml_dtypes

    g = np.zeros((128, 254), dtype=ml_dtypes.bfloat16)
    for p in range(128):
        g[p, 126 + (p // 64)] = 1.0
    return g


def _build():
    import concourse.bass as bass
    import concourse.tile as tile
    from concourse import bacc, mybir

    nc = bacc.Bacc(
        "TRN2", target_bir_lowering=False, debug=False, num_devices=N_CORES
    )
    f32 = mybir.dt.float32
    bf16 = mybir.dt.bfloat16

    x_dram = nc.dram_tensor(
        "x", [B_PER_CORE, F, E], f32, kind="ExternalInput"
    ).ap()
    g_dram = nc.dram_tensor("g", [128, 254], bf16, kind="ExternalInput").ap()
    out_dram = nc.dram_tensor("out", [128, J], f32, kind="ExternalOutput").ap()

    # b = 16*t + 2*j + p  ->  partition (p f), free (k j e).
    # The composite partition index p*64+f maps to DRAM element offset
    # p*F*E + f*E = (p*64 + f)*64 — a uniform stride of 64, so one AP dim.
    def x_super_ap(s: int) -> bass.AP:
        return bass.AP(
            tensor=x_dram.tensor,
            offset=s * SUPER * 16 * F * E,
            ap=[
                [E, 128],           # partition: (b_hi, f), stride E=64
                [16 * F * E, SUPER],  # k: tile within superblock (16 b's)
                [2 * F * E, J],     # j: b-pair
                [1, E],             # e
            ],
        )

    with tile.TileContext(nc) as tc:
        with (
            tc.tile_pool(name="const", bufs=1) as const_pool,
            tc.tile_pool(name="xin", bufs=6) as x_pool,
            tc.tile_pool(name="xsq", bufs=3) as x2_pool,
            tc.tile_pool(name="xcu", bufs=3) as x3_pool,
            tc.tile_pool(name="acc", bufs=1, space="PSUM") as psum_pool,
            tc.tile_pool(name="tail", bufs=1) as tail_pool,
        ):
            g_sb = const_pool.tile([128, 254], bf16)
            nc.sync.dma_start(out=g_sb[:], in_=g_dram[:])

            p1 = psum_pool.tile([128, FD], f32)
            p2 = psum_pool.tile([128, FD], f32)
            p3 = psum_pool.tile([128, FD], f32)

            for s in range(N_SUPER):
                xb = x_pool.tile([128, SFD], bf16)
                # SWDGE cast-DMA: fp32 in DRAM -> bf16 in SBUF
                nc.gpsimd.dma_start(out=xb[:], in_=x_super_ap(s))
                x2b = x2_pool.tile([128, SFD], bf16)
                nc.scalar.square(x2b[:], xb[:])
                x3b = x3_pool.tile([128, SFD], bf16)
                nc.vector.tensor_mul(x3b[:], x2b[:], xb[:])
                for k in range(SUPER):
                    t = s * SUPER + k
                    lhsT = g_sb[:, 126 - 2 * t : 254 - 2 * t]
                    first = t == 0
                    last = t == TILES - 1
                    sl = slice(k * FD, (k + 1) * FD)
                    for psum, src in ((p1, xb), (p2, x2b), (p3, x3b)):
                        nc.tensor.matmul(
                            psum[:],
                            lhsT,
                            src[:, sl],
                            start=first,
                            stop=last,
                            skip_group_check=True,
                        )

            # finale: e3 = (p1^3 - 3 p1 p2 + 2 p3) / 6, summed over e
            t1 = tail_pool.tile([128, FD], f32)
            nc.scalar.square(t1[:], p1[:])  # p1^2
            u2 = tail_pool.tile([128, FD], f32)
            nc.vector.scalar_tensor_tensor(  # p1^2 - 3 p2
                u2[:], p2[:], -3.0, t1[:],
                op0=mybir.AluOpType.mult, op1=mybir.AluOpType.add,
            )
            u3 = tail_pool.tile([128, FD], f32)
            nc.vector.tensor_mul(u3[:], u2[:], p1[:])  # p1^3 - 3 p1 p2
            u5 = tail_pool.tile([128, FD], f32)
            nc.vector.scalar_tensor_tensor(  # + 2 p3
                u5[:], p3[:], 2.0, u3[:],
                op0=mybir.AluOpType.mult, op1=mybir.AluOpType.add,
            )
            red = tail_pool.tile([128, J], f32)
            nc.vector.reduce_sum(
                red[:],
                u5[:].rearrange("p (j e) -> p j e", j=J),
                axis=mybir.AxisListType.X,
            )
            outt = tail_pool.tile([128, J], f32)
            nc.vector.tensor_scalar_mul(outt[:], red[:], 1.0 / 6.0)
            nc.sync.dma_start(out=out_dram[:], in_=outt[:])

    nc.compile()
    return nc


def _get_nc():
    if "nc" not in _cache:
        _cache["nc"] = _build()
    return _cache["nc"]


def _unpermute(r: np.ndarray) -> np.ndarray:
    # r[(2*tau + b_hi), j] is the value for b_local = 16*tau + 2*j + b_hi
    return np.transpose(r.reshape(TILES, 2, J), (0, 2, 1)).reshape(-1)


def _run(x: np.ndarray, **kwargs):
    from concourse.bass_utils import run_bass_kernel_spmd

    nc = _get_nc()
    g = _make_g()
    shards = x.reshape(N_CORES, B_PER_CORE, F, E)
    in_maps = [
        {"x": np.ascontiguousarray(shards[c]), "g": g} for c in range(N_CORES)
    ]
    res = run_bass_kernel_spmd(nc, in_maps, core_ids=list(range(N_CORES)), **kwargs)
    out = np.concatenate(
        [_unpermute(np.asarray(res.results[c]["out"])) for c in range(N_CORES)]
    ).astype(np.float32)
    return out, res


def kernel(**inputs) -> np.ndarray:
    x = np.ascontiguousarray(np.asarray(inputs["x"], dtype=np.float32))
    assert x.shape == (B, F, E), x.shape
    out, _ = _run(x)
    return out
